# revision 8
# baseline (speedup 1.0000x reference)
"""Trainium2 Bass kernel for nn_AttentionModule (conv3x3 -> BN -> LeakyReLU ->
spatial attention -> residual -> LN -> LeakyReLU).

Math: softmax(k, axis=N).sum(axis=N) == 1, so the q/k branches and both
softmaxes are dead; the module reduces to
    x   = leaky(BN(conv3x3(inputs)))        # batch-stat BN, eps=1e-3
    y   = conv1x1(x, wv + I) + bv           # residual folded into weights
    out = leaky(LN(y))                      # per-sample LN, eps=1e-3
(cbl_b cancels inside train-mode BN; wq/bq/wk/bk are dead.)

Sharding: data-parallel, 2 images/core on 8 cores; per-channel BN (mean,
E[x^2]) goes through one small AllReduce per 128-channel chunk, the first
fully hidden under the second chunk's convolution.

v2 layout/schedule (vs the 190us baseline):
 - All DMAs use flat per-partition-contiguous APs (1 big descriptor per
   partition instead of per-row 264B descriptors) and are spread over the
   sync/vector/scalar/gpsimd engine queues, so the first conv matmul can
   issue at ~5us instead of ~17us.
 - ACT sqrt table set is preloaded at t=0 by a dummy op (the sqrt set also
   carries Copy/Identity/Prelu, so no mid-kernel table switches).
 - conv3x3 accumulates into [128,2048] PSUM tiles (4 banks); BN stats are
   taken directly from PSUM by DVE while ACT drains PSUM->X in one
   2048-wide activation per group.
 - BN apply is done in-place on X (Prelu with per-channel scale/bias); its
   accum_out gives per-channel running sums of leaky(BN(x)) for free, from
   which the LN *mean* is recovered analytically:
     sum(Y) = sum_i colsum(wv_eff)_i * rowsum(xb)_i   (bv=0 fast path).
 - Phase 2 never materializes y in SBUF: conv1x1 runs twice. Pass 1 feeds
   tensor_tensor_reduce (sum of Y^2 straight from PSUM); pass 2 re-runs the
   matmuls and fuses bias+LN+leaky into a single PSUM->SBUF activation per
   1024-pixel block, which streams out over two DMA queues.
 - The chunk-1 AllReduce latency is partially hidden behind kc0-prefetch
   matmuls for the first two blocks plus the BN0 apply.
"""

import numpy as np

import concourse.bacc as bacc
import concourse.tile as tile
from concourse import mybir
from concourse.bass_utils import run_bass_kernel_spmd

B, H, W, CIN, C = 16, 64, 64, 128, 256
NCORES = 8
BL = B // NCORES            # images per core
HP, WP = H + 2, W + 2       # padded spatial dims
PIX = BL * H * W            # pixels per core (8192)
IPIX = H * W                # pixels per image (4096)
EPS = 1e-3
F32 = mybir.dt.float32
F32R = mybir.dt.float32r
AF = mybir.ActivationFunctionType
OP = mybir.AluOpType

ALPHA = 0.3                 # LeakyReLU slope
NBLK = 8                    # phase-2 blocks of 1024 px (2048 psum elems)
BPX = PIX // NBLK           # 1024 pixels per block

_CACHE = {}
LAST_RESULT = None


def _build(fast_ln: bool):
    nc = bacc.Bacc("TRN2", num_devices=NCORES)

    xin = nc.dram_tensor("xin", [CIN, BL * HP * WP], F32R, kind="ExternalInput")
    cw = nc.dram_tensor("cw", [CIN, 2 * 9 * 128], F32R, kind="ExternalInput")
    wvd = nc.dram_tensor("wvd", [128, 2 * 2 * 128], F32R, kind="ExternalInput")
    # per-channel params: g0,g1,b0,b1,bv0,bv1,cs0,cs1 (cs = colsum of wv_eff)
    bnp = nc.dram_tensor("bnp", [128, 8], F32, kind="ExternalInput")
    if not fast_ln:
        lng = nc.dram_tensor("lng", [C, IPIX], F32, kind="ExternalInput")
        lnb = nc.dram_tensor("lnb", [C, IPIX], F32, kind="ExternalInput")
    yout = nc.dram_tensor("yout", [128, NBLK * 2048], F32, kind="ExternalOutput")
    cc_in = [nc.dram_tensor(f"cc_in{ch}", [128, 2], F32) for ch in range(2)]
    cc_out = [nc.dram_tensor(f"cc_out{ch}", [128, 2], F32, addr_space="Shared")
              for ch in range(2)]

    with tile.TileContext(nc) as tc:
        with tc.tile_pool(name="wpool", bufs=1) as wpool, \
             tc.tile_pool(name="stat", bufs=1) as stat, \
             tc.tile_pool(name="Xp", bufs=2) as Xp, \
             tc.tile_pool(name="outp", bufs=3) as outp, \
             tc.tile_pool(name="ps", bufs=2, space="PSUM") as ps:

            # ---------------- startup: weights + input DMAs, table preload ----
            wt = wpool.tile([CIN, 2, 9, 128], F32R, tag="wt")
            wvt = wpool.tile([128, 2, 2, 128], F32R, tag="wvt")
            bnpt = stat.tile([128, 8], F32, tag="bnpt")
            wtf = wt[:].rearrange("k c t m -> k (c t m)")
            cwf = cw.ap()[:]
            # scalar queue: weights (chunk0 first -- gates the first matmul)
            nc.scalar.dma_start(out=wtf[:, 0:1152], in_=cwf[:, 0:1152])
            nc.scalar.dma_start(out=wtf[:, 1152:2304], in_=cwf[:, 1152:2304])
            nc.scalar.dma_start(out=wvt[:].rearrange("k a b m -> k (a b m)"),
                                in_=wvd.ap()[:])
            nc.scalar.dma_start(out=bnpt[:], in_=bnp.ap()[:])

            eps128 = stat.tile([128, 1], F32, tag="eps128")
            onesM = stat.tile([128, 128], F32, tag="onesM")
            scr1 = stat.tile([128, 1], F32, tag="scr1")
            nc.vector.memset(eps128[:], EPS)
            nc.vector.memset(onesM[:], 1.0)
            # preload the sqrt table set (it also carries Copy/Identity/Prelu)
            nc.scalar.activation(out=scr1[:], in_=eps128[:], func=AF.Sqrt)

            # input: 4 flat pieces in consumption order over two queues
            xt = wpool.tile([CIN, BL * HP * WP], F32R, tag="xt")
            xinf = xin.ap()[:]
            half = 34 * WP                      # rows 0:34 of an image plane
            plane = HP * WP
            nc.sync.dma_start(out=xt[:, 0:half], in_=xinf[:, 0:half])
            nc.sync.dma_start(out=xt[:, half:plane], in_=xinf[:, half:plane])
            nc.gpsimd.dma_start(out=xt[:, plane:plane + half],
                                in_=xinf[:, plane:plane + half])
            nc.gpsimd.dma_start(out=xt[:, plane + half:2 * plane],
                                in_=xinf[:, plane + half:2 * plane])
            xtv = xt[:].rearrange("k (b h w) -> k b h w", b=BL, h=HP)

            X = [Xp.tile([128, PIX], F32R, tag="X", name=f"X{i}") for i in range(2)]
            if not fast_ln:
                lngt = wpool.tile([128, 2, IPIX], F32, tag="lngt")
                lnbt = wpool.tile([128, 2, IPIX], F32, tag="lnbt")
                for ch in range(2):
                    nc.sync.dma_start(out=lngt[:, ch, :],
                                      in_=lng.ap()[ch * 128:(ch + 1) * 128, :])
                    nc.sync.dma_start(out=lnbt[:, ch, :],
                                      in_=lnb.ap()[ch * 128:(ch + 1) * 128, :])

            # ---------------- stats / coef tiles ----------------------------
            bnstat = stat.tile([128, 2, 4, 4, 6], F32, tag="bnstat")
            mv = stat.tile([128, 2, 2], F32, tag="mv")       # (mean, E2) per ch
            gsum = stat.tile([128, 2, 2], F32, tag="gsum")   # AR result
            tmp = stat.tile([128, 2, 2], F32, tag="tmpbn")
            sbn = stat.tile([128, 2], F32, tag="sbn")        # BN scale per ch
            bbn = stat.tile([128, 2], F32, tag="bbn")        # BN bias per ch
            rhsT = stat.tile([128, 2, 2], F32, tag="rhsT")   # per img: (SM, SE2)
            # LN stats records: [img, ch, blk-in-img, 512-slice, 6]
            lnstat = stat.tile([128, 2, 2, 4, 2, 6], F32, tag="lnstat")
            mvb = stat.tile([128, 2], F32, tag="mvb")
            mE = stat.tile([128, 2, 2], F32, tag="mE")       # per img (m, E2)
            rr = stat.tile([128, 2], F32, tag="rr")          # per img rstd
            lbias = stat.tile([128, 2, 2], F32, tag="lbias") # per (img, ch) bias

            def conv_group(ch, g):
                P = ps.tile([128, 2048], F32, tag="ps", name=f"cv{ch}_{g}")
                b, half_g = g // 2, g % 2
                for tap in range(9):
                    dy, dx = tap // 3, tap % 3
                    lhsT = wt[:, ch, tap, :]
                    for sl in range(4):
                        r0 = half_g * 32 + sl * 8
                        rhs = xtv[:, b, r0 + dy:r0 + dy + 8, dx:dx + W]
                        nc.tensor.matmul(P[:, sl * 512:(sl + 1) * 512], lhsT, rhs,
                                         start=(tap == 0), stop=(tap == 8))
                for sl in range(4):
                    nc.vector.bn_stats(out=bnstat[:, ch, g, sl, :],
                                       in_=P[:, sl * 512:(sl + 1) * 512])
                nc.scalar.activation(out=X[ch][:, g * 2048:(g + 1) * 2048],
                                     in_=P[:], func=AF.Copy)

            def bn_reduce_and_allreduce(ch):
                nc.vector.bn_aggr(out=mv[:, ch, :], in_=bnstat[:, ch])
                mean, var = mv[:, ch, 0:1], mv[:, ch, 1:2]
                # E2 = mean^2 + var (AllReduce of means/E2 is exact: equal counts)
                nc.vector.tensor_scalar(var, mean, mean, var, OP.mult, OP.add)
                nc.gpsimd.dma_start(out=cc_in[ch].ap()[:], in_=mv[:, ch, :])
                nc.gpsimd.collective_compute(
                    "AllReduce", OP.add, replica_groups=[list(range(NCORES))],
                    ins=[cc_in[ch].ap()[:]], outs=[cc_out[ch].ap()[:]])
                nc.gpsimd.dma_start(out=gsum[:, ch, :], in_=cc_out[ch].ap()[:])

            def bn_coefs(ch):
                mu, ex2 = tmp[:, ch, 0:1], tmp[:, ch, 1:2]
                nc.vector.tensor_scalar_mul(mu, gsum[:, ch, 0:1], 1.0 / NCORES)
                nc.vector.tensor_scalar_mul(ex2, gsum[:, ch, 1:2], 1.0 / NCORES)
                var = sbn[:, ch:ch + 1]
                nc.vector.tensor_scalar(var, mu, mu, None, OP.mult)
                nc.vector.tensor_sub(var, ex2, var)
                nc.scalar.activation(out=var, in_=var, func=AF.Sqrt, bias=eps128[:])
                nc.vector.reciprocal(out=var, in_=var)
                nc.vector.tensor_mul(var, var, bnpt[:, ch:ch + 1])      # * gamma
                nc.vector.tensor_mul(mu, mu, var)
                nc.vector.tensor_sub(bbn[:, ch:ch + 1], bnpt[:, 2 + ch:3 + ch], mu)

            def bn_apply_piece(ch, p):
                seg = X[ch][:, p * 2048:(p + 1) * 2048]
                nc.scalar.activation(out=seg, in_=seg, func=AF.Prelu,
                                     bias=bbn[:, ch:ch + 1], scale=sbn[:, ch:ch + 1],
                                     alpha=ALPHA)

            # ---------------- phase 1: conv3x3 + BN stats --------------------
            for g in range(4):
                conv_group(0, g)
            bn_reduce_and_allreduce(0)
            bn_coefs(0)
            for g in range(4):
                conv_group(1, g)
                bn_apply_piece(0, g)    # ACT: interleaves with chunk-1 copies
            bn_reduce_and_allreduce(1)
            bn_coefs(1)
            for p in range(4):
                bn_apply_piece(1, p)

            # ---------------- phase 2: conv1x1 x2, LN, finals ----------------
            accs = {}

            def kc_mms(dst, blk, kc, start, stop):
                for ch in range(2):
                    lhsT = wvt[:, kc, ch, :]
                    for sl in range(2):
                        rhs = X[kc][:, blk * BPX + sl * 512:blk * BPX + (sl + 1) * 512]
                        nc.tensor.matmul(dst[:, ch * 1024 + sl * 512:
                                             ch * 1024 + (sl + 1) * 512],
                                         lhsT, rhs, start=start, stop=stop)

            def p1_stats(blk):
                img = blk // 4
                for sl in range(4):
                    nc.vector.bn_stats(
                        out=lnstat[:, img, sl // 2, blk % 4, sl % 2, :],
                        in_=accs[blk][:, sl * 512:(sl + 1) * 512])

            def img_combine_pre(img):
                """DVE part: rhsT[:, img, :] = per-partition (Σmean', ΣE2')."""
                u = rhsT[:, img, 0:1]
                s2 = rhsT[:, img, 1:2]
                # mean'_ch = mean_ch + bv_ch ; E2'_ch = var_ch + mean'^2
                for ch in range(2):
                    nc.vector.bn_aggr(out=mvb[:], in_=lnstat[:, img, ch])
                    mm, vv = mvb[:, 0:1], mvb[:, 1:2]
                    nc.vector.tensor_add(mm, mm, bnpt[:, 4 + ch:5 + ch])
                    nc.vector.tensor_scalar(vv, mm, mm, vv, OP.mult, OP.add)
                    if ch == 0:
                        nc.vector.tensor_copy(u, mm)
                        nc.vector.tensor_copy(s2, vv)
                    else:
                        nc.vector.tensor_add(u, u, mm)
                        nc.vector.tensor_add(s2, s2, vv)

            def img_pcomb(img):
                """PE: reduce rhsT across partitions (broadcast to all)."""
                pc = ps.tile([128, 2048], F32, tag="ps", name=f"pcomb{img}")
                nc.tensor.matmul(pc[:, 0:2], onesM[:], rhsT[:, img, :],
                                 start=True, stop=True)
                nc.vector.tensor_scalar(mE[:, img, :], pc[:, 0:2], 1.0 / C,
                                        None, OP.mult)

            def img_coefs(img):
                m, e2 = mE[:, img, 0:1], mE[:, img, 1:2]
                v = rr[:, img:img + 1]
                nc.vector.tensor_scalar(v, m, m, None, OP.mult)
                nc.vector.tensor_sub(v, e2, v)
                nc.scalar.activation(out=v, in_=v, func=AF.Sqrt, bias=eps128[:])
                nc.vector.reciprocal(out=v, in_=v)              # r = rstd
                # bias per (img, ch) = r * (bv_ch - m)
                for ch in range(2):
                    bb = lbias[:, img, ch:ch + 1]
                    nc.vector.tensor_sub(bb, bnpt[:, 4 + ch:5 + ch], m)
                    nc.vector.tensor_mul(bb, bb, v)

            def p2_block(blk):
                img = blk // 4
                P = ps.tile([128, 2048], F32, tag="ps", name=f"p2_{blk}")
                kc_mms(P, blk, 0, True, False)
                kc_mms(P, blk, 1, False, True)
                ot = outp.tile([128, 2048], F32, tag="ot", name=f"ot{blk}")
                if fast_ln:
                    for ch in range(2):
                        nc.scalar.activation(
                            out=ot[:, ch * 1024:(ch + 1) * 1024],
                            in_=P[:, ch * 1024:(ch + 1) * 1024],
                            func=AF.Prelu, bias=lbias[:, img, ch:ch + 1],
                            scale=rr[:, img:img + 1], alpha=ALPHA)
                else:
                    lo = (blk % 4) * BPX
                    for ch in range(2):
                        seg = ot[:, ch * 1024:(ch + 1) * 1024]
                        nc.scalar.activation(
                            out=seg, in_=P[:, ch * 1024:(ch + 1) * 1024],
                            func=AF.Identity, bias=lbias[:, img, ch:ch + 1],
                            scale=rr[:, img:img + 1])
                        nc.vector.tensor_mul(seg, seg, lngt[:, ch, lo:lo + BPX])
                        nc.vector.tensor_add(seg, seg, lnbt[:, ch, lo:lo + BPX])
                        nc.scalar.activation(out=seg, in_=seg, func=AF.Prelu,
                                             bias=0.0, scale=1.0, alpha=ALPHA)
                q = nc.sync if blk % 2 == 0 else nc.gpsimd
                q.dma_start(out=yout.ap()[:, blk * 2048:(blk + 1) * 2048], in_=ot[:])

            # pass 1 with kc0 prefetch for blocks 0..1 (hides part of the AR)
            for blk in range(2):
                accs[blk] = ps.tile([128, 2048], F32, tag="ps", name=f"p1_{blk}")
                kc_mms(accs[blk], blk, 0, True, False)
            for blk in range(NBLK):
                if blk >= 2:
                    accs[blk] = ps.tile([128, 2048], F32, tag="ps", name=f"p1_{blk}")
                    kc_mms(accs[blk], blk, 0, True, False)
                kc_mms(accs[blk], blk, 1, False, True)
                p1_stats(blk)
                if blk == 3:
                    img_combine_pre(0)
                if blk == 5:
                    img_pcomb(0)
                    img_coefs(0)
            img_combine_pre(1)
            img_pcomb(1)
            img_coefs(1)

            # pass 2: re-run matmuls, fuse LN+leaky straight out of PSUM
            for blk in range(NBLK):
                p2_block(blk)

    nc.compile()
    return nc


def kernel(**inputs):
    global LAST_RESULT
    x = np.ascontiguousarray(np.asarray(inputs["inputs"], dtype=np.float32))
    cbl_w = np.asarray(inputs["cbl_w"], dtype=np.float32)
    bn_gamma = np.asarray(inputs["bn_gamma"], dtype=np.float32)
    bn_beta = np.asarray(inputs["bn_beta"], dtype=np.float32)
    wv = np.asarray(inputs["wv"], dtype=np.float32).reshape(C, C)
    bv = np.asarray(inputs["bv"], dtype=np.float32)
    ln_gamma = np.asarray(inputs["ln_gamma"], dtype=np.float32)
    ln_beta = np.asarray(inputs["ln_beta"], dtype=np.float32)

    fast_ln = bool(np.all(ln_gamma == 1.0) and np.all(ln_beta == 0.0))
    # host-side repack (free for HW time): channel-major, pre-padded input
    xp = np.zeros((NCORES, CIN, BL, HP, WP), np.float32)
    xp[:, :, :, 1:H + 1, 1:W + 1] = (
        x.reshape(NCORES, BL, H, W, CIN).transpose(0, 4, 1, 2, 3))
    xin = np.ascontiguousarray(xp.reshape(NCORES, CIN, BL * HP * WP))
    # conv weights chunk-major: [cin, ch, tap, m]
    cw = np.ascontiguousarray(
        cbl_w.reshape(9, CIN, 2, 128).transpose(1, 2, 0, 3).reshape(CIN, 2304))
    wv_eff = wv + np.eye(C, dtype=np.float32)
    # [i_local, kc, ch, m]
    wvd = np.ascontiguousarray(
        wv_eff.reshape(2, 128, 2, 128).transpose(1, 0, 2, 3).reshape(128, 512))
    colsum = wv_eff.sum(axis=1)          # [256]
    bnp = np.ascontiguousarray(np.stack([
        bn_gamma[0:128], bn_gamma[128:256],
        bn_beta[0:128], bn_beta[128:256],
        bv[0:128], bv[128:256],
        colsum[0:128], colsum[128:256]], axis=1))

    if fast_ln not in _CACHE:
        _CACHE[fast_ln] = _build(fast_ln)
    nc = _CACHE[fast_ln]

    in_maps = []
    for i in range(NCORES):
        m = {"xin": xin[i], "cw": cw, "wvd": wvd, "bnp": bnp}
        if not fast_ln:
            m["lng"] = np.ascontiguousarray(
                ln_gamma.transpose(2, 0, 1).reshape(C, IPIX))
            m["lnb"] = np.ascontiguousarray(
                ln_beta.transpose(2, 0, 1).reshape(C, IPIX))
        in_maps.append(m)

    res = run_bass_kernel_spmd(nc, in_maps, core_ids=list(range(NCORES)))
    LAST_RESULT = res

    out = np.empty((B, H, W, C), np.float32)
    for i in range(NCORES):
        yc = res.results[i]["yout"].reshape(128, 2, 4, 2, 1024)
        # axes: [p, img, blk4, ch, j] -> [img, blk4, j, ch, p]
        img = yc.transpose(1, 2, 4, 3, 0).reshape(BL, H, W, C)
        out[i * BL:(i + 1) * BL] = img
    return out


# revision 9
# speedup vs baseline: 1.0913x; 1.0913x over previous
"""Trainium2 Bass kernel for nn_AttentionModule (conv3x3 -> BN -> LeakyReLU ->
spatial attention -> residual -> LN -> LeakyReLU).

Math: softmax(k, axis=N).sum(axis=N) == 1, so the q/k branches and both
softmaxes are dead; the module reduces to
    x   = leaky(BN(conv3x3(inputs)))        # batch-stat BN, eps=1e-3
    y   = conv1x1(x, wv + I) + bv           # residual folded into weights
    out = leaky(LN(y))                      # per-sample LN, eps=1e-3
(cbl_b cancels inside train-mode BN; wq/bq/wk/bk are dead.)

Sharding: data-parallel, 2 images/core on 8 cores; per-channel BN (mean,
E[x^2]) goes through one small AllReduce per 128-channel chunk, the first
fully hidden under the second chunk's convolution.

v2 layout/schedule (vs the 190us baseline):
 - All DMAs use flat per-partition-contiguous APs (1 big descriptor per
   partition instead of per-row 264B descriptors) and are spread over the
   sync/vector/scalar/gpsimd engine queues, so the first conv matmul can
   issue at ~5us instead of ~17us.
 - ACT sqrt table set is preloaded at t=0 by a dummy op (the sqrt set also
   carries Copy/Identity/Prelu, so no mid-kernel table switches).
 - conv3x3 accumulates into [128,2048] PSUM tiles (4 banks); BN stats are
   taken directly from PSUM by DVE while ACT drains PSUM->X in one
   2048-wide activation per group.
 - BN apply is done in-place on X (Prelu with per-channel scale/bias); its
   accum_out gives per-channel running sums of leaky(BN(x)) for free, from
   which the LN *mean* is recovered analytically:
     sum(Y) = sum_i colsum(wv_eff)_i * rowsum(xb)_i   (bv=0 fast path).
 - Phase 2 never materializes y in SBUF: conv1x1 runs twice. Pass 1 feeds
   tensor_tensor_reduce (sum of Y^2 straight from PSUM); pass 2 re-runs the
   matmuls and fuses bias+LN+leaky into a single PSUM->SBUF activation per
   1024-pixel block, which streams out over two DMA queues.
 - The chunk-1 AllReduce latency is partially hidden behind kc0-prefetch
   matmuls for the first two blocks plus the BN0 apply.
"""

import numpy as np

import concourse.bacc as bacc
import concourse.tile as tile
from concourse import mybir
from concourse.bass_utils import run_bass_kernel_spmd

B, H, W, CIN, C = 16, 64, 64, 128, 256
NCORES = 8
BL = B // NCORES            # images per core
HP, WP = H + 2, W + 2       # padded spatial dims
PIX = BL * H * W            # pixels per core (8192)
IPIX = H * W                # pixels per image (4096)
EPS = 1e-3
F32 = mybir.dt.float32
F32R = mybir.dt.float32r
AF = mybir.ActivationFunctionType
OP = mybir.AluOpType

ALPHA = 0.3                 # LeakyReLU slope
NBLK = 8                    # phase-2 blocks of 1024 px (2048 psum elems)
BPX = PIX // NBLK           # 1024 pixels per block

_CACHE = {}
LAST_RESULT = None


def _build(fast_ln: bool):
    nc = bacc.Bacc("TRN2", num_devices=NCORES)

    xin = nc.dram_tensor("xin", [CIN, BL * HP * WP], F32R, kind="ExternalInput")
    cw = nc.dram_tensor("cw", [CIN, 2 * 9 * 128], F32R, kind="ExternalInput")
    wvd = nc.dram_tensor("wvd", [128, 2 * 2 * 128], F32R, kind="ExternalInput")
    # per-channel params: g0,g1,b0,b1,bv0,bv1,cs0,cs1 (cs = colsum of wv_eff)
    bnp = nc.dram_tensor("bnp", [128, 8], F32, kind="ExternalInput")
    if not fast_ln:
        lng = nc.dram_tensor("lng", [C, IPIX], F32, kind="ExternalInput")
        lnb = nc.dram_tensor("lnb", [C, IPIX], F32, kind="ExternalInput")
    yout = nc.dram_tensor("yout", [128, NBLK * 2048], F32, kind="ExternalOutput")
    cc_in = [nc.dram_tensor(f"cc_in{ch}", [128, 2], F32) for ch in range(2)]
    cc_out = [nc.dram_tensor(f"cc_out{ch}", [128, 2], F32, addr_space="Shared")
              for ch in range(2)]

    with tile.TileContext(nc) as tc:
        with tc.tile_pool(name="wpool", bufs=1) as wpool, \
             tc.tile_pool(name="stat", bufs=1) as stat, \
             tc.tile_pool(name="Xp", bufs=2) as Xp, \
             tc.tile_pool(name="outp", bufs=3) as outp, \
             tc.tile_pool(name="ps", bufs=2, space="PSUM") as ps:

            # ---------------- startup: weights + input DMAs, table preload ----
            wt = wpool.tile([CIN, 2, 9, 128], F32R, tag="wt")
            wvt = wpool.tile([128, 2, 2, 128], F32R, tag="wvt")
            bnpt = stat.tile([128, 8], F32, tag="bnpt")
            wtf = wt[:].rearrange("k c t m -> k (c t m)")
            cwf = cw.ap()[:]
            xt = wpool.tile([CIN, BL * HP * WP], F32R, tag="xt")
            xinf = xin.ap()[:]
            piece = 34 * WP                     # rows 0:34 of an image plane
            plane = HP * WP
            # sync queue boots first: chunk-0 weights + the rows the first
            # conv group needs, then the rest of image 0.
            nc.sync.dma_start(out=wtf[:, 0:1152], in_=cwf[:, 0:1152])
            nc.sync.dma_start(out=xt[:, 0:piece], in_=xinf[:, 0:piece])
            nc.sync.dma_start(out=xt[:, piece:plane], in_=xinf[:, piece:plane])
            # gpsimd queue: image 1 + chunk-1 weights (needed at ~50%)
            nc.gpsimd.dma_start(out=xt[:, plane:plane + piece],
                                in_=xinf[:, plane:plane + piece])
            nc.gpsimd.dma_start(out=xt[:, plane + piece:2 * plane],
                                in_=xinf[:, plane + piece:2 * plane])
            nc.gpsimd.dma_start(out=wtf[:, 1152:2304], in_=cwf[:, 1152:2304])
            # scalar queue: small params, then the table preload
            nc.scalar.dma_start(out=wvt[:].rearrange("k a b m -> k (a b m)"),
                                in_=wvd.ap()[:])
            nc.scalar.dma_start(out=bnpt[:], in_=bnp.ap()[:])
            xtv = xt[:].rearrange("k (b h w) -> k b h w", b=BL, h=HP)

            eps128 = stat.tile([128, 1], F32, tag="eps128")
            onesM = stat.tile([128, 128], F32, tag="onesM")
            scr1 = stat.tile([128, 1], F32, tag="scr1")
            nc.vector.memset(eps128[:], EPS)
            nc.vector.memset(onesM[:], 1.0)
            # preload the sqrt table set (it also carries Copy/Identity/Prelu)
            nc.scalar.activation(out=scr1[:], in_=eps128[:], func=AF.Sqrt)

            X = [Xp.tile([128, PIX], F32R, tag="X", name=f"X{i}") for i in range(2)]
            if not fast_ln:
                lngt = wpool.tile([128, 2, IPIX], F32, tag="lngt")
                lnbt = wpool.tile([128, 2, IPIX], F32, tag="lnbt")
                for ch in range(2):
                    nc.sync.dma_start(out=lngt[:, ch, :],
                                      in_=lng.ap()[ch * 128:(ch + 1) * 128, :])
                    nc.sync.dma_start(out=lnbt[:, ch, :],
                                      in_=lnb.ap()[ch * 128:(ch + 1) * 128, :])

            # ---------------- stats / coef tiles ----------------------------
            bnstat = stat.tile([128, 2, 4, 4, 6], F32, tag="bnstat")
            mv = stat.tile([128, 2, 2], F32, tag="mv")       # (mean, E2) per ch
            gsum = stat.tile([128, 2, 2], F32, tag="gsum")   # AR result
            tmp = stat.tile([128, 2, 2], F32, tag="tmpbn")
            sbn = stat.tile([128, 2], F32, tag="sbn")        # BN scale per ch
            bbn = stat.tile([128, 2], F32, tag="bbn")        # BN bias per ch
            rhsT = stat.tile([128, 2, 2], F32, tag="rhsT")   # per img: (SM, SE2)
            # LN stats records: [img, ch, blk-in-img, 512-slice, 6]
            lnstat = stat.tile([128, 2, 2, 4, 2, 6], F32, tag="lnstat")
            mvb = stat.tile([128, 2], F32, tag="mvb")
            mE = stat.tile([128, 2, 2], F32, tag="mE")       # per img (m, E2)
            rr = stat.tile([128, 2], F32, tag="rr")          # per img rstd
            lbias = stat.tile([128, 2, 2], F32, tag="lbias") # per (img, ch) bias

            def conv_group(ch, g):
                P = ps.tile([128, 2048], F32, tag="ps", name=f"cv{ch}_{g}")
                b, half_g = g // 2, g % 2
                for tap in range(9):
                    dy, dx = tap // 3, tap % 3
                    lhsT = wt[:, ch, tap, :]
                    for sl in range(4):
                        r0 = half_g * 32 + sl * 8
                        rhs = xtv[:, b, r0 + dy:r0 + dy + 8, dx:dx + W]
                        nc.tensor.matmul(P[:, sl * 512:(sl + 1) * 512], lhsT, rhs,
                                         start=(tap == 0), stop=(tap == 8))
                for sl in range(4):
                    nc.vector.bn_stats(out=bnstat[:, ch, g, sl, :],
                                       in_=P[:, sl * 512:(sl + 1) * 512])
                nc.scalar.activation(out=X[ch][:, g * 2048:(g + 1) * 2048],
                                     in_=P[:], func=AF.Copy)

            def bn_reduce_and_allreduce(ch):
                nc.vector.bn_aggr(out=mv[:, ch, :], in_=bnstat[:, ch])
                mean, var = mv[:, ch, 0:1], mv[:, ch, 1:2]
                # E2 = mean^2 + var (AllReduce of means/E2 is exact: equal counts)
                nc.vector.tensor_scalar(var, mean, mean, var, OP.mult, OP.add)
                nc.gpsimd.dma_start(out=cc_in[ch].ap()[:], in_=mv[:, ch, :])
                nc.gpsimd.collective_compute(
                    "AllReduce", OP.add, replica_groups=[list(range(NCORES))],
                    ins=[cc_in[ch].ap()[:]], outs=[cc_out[ch].ap()[:]])
                nc.gpsimd.dma_start(out=gsum[:, ch, :], in_=cc_out[ch].ap()[:])

            def bn_coefs(ch):
                mu, ex2 = tmp[:, ch, 0:1], tmp[:, ch, 1:2]
                nc.vector.tensor_scalar_mul(mu, gsum[:, ch, 0:1], 1.0 / NCORES)
                nc.vector.tensor_scalar_mul(ex2, gsum[:, ch, 1:2], 1.0 / NCORES)
                var = sbn[:, ch:ch + 1]
                nc.vector.tensor_scalar(var, mu, mu, None, OP.mult)
                nc.vector.tensor_sub(var, ex2, var)
                nc.scalar.activation(out=var, in_=var, func=AF.Sqrt, bias=eps128[:])
                nc.vector.reciprocal(out=var, in_=var)
                nc.vector.tensor_mul(var, var, bnpt[:, ch:ch + 1])      # * gamma
                nc.vector.tensor_mul(mu, mu, var)
                nc.vector.tensor_sub(bbn[:, ch:ch + 1], bnpt[:, 2 + ch:3 + ch], mu)

            def bn_apply_piece(ch, p):
                seg = X[ch][:, p * 2048:(p + 1) * 2048]
                nc.scalar.activation(out=seg, in_=seg, func=AF.Prelu,
                                     bias=bbn[:, ch:ch + 1], scale=sbn[:, ch:ch + 1],
                                     alpha=ALPHA)

            # ---------------- phase 1: conv3x3 + BN stats --------------------
            for g in range(4):
                conv_group(0, g)
            bn_reduce_and_allreduce(0)
            bn_coefs(0)
            for g in range(4):
                conv_group(1, g)
                bn_apply_piece(0, g)    # ACT: interleaves with chunk-1 copies
            bn_reduce_and_allreduce(1)
            bn_coefs(1)
            for p in range(4):
                bn_apply_piece(1, p)

            # ---------------- phase 2: conv1x1 x2, LN, finals ----------------
            accs = {}

            def kc_mms(dst, blk, kc, start, stop):
                for ch in range(2):
                    lhsT = wvt[:, kc, ch, :]
                    for sl in range(2):
                        rhs = X[kc][:, blk * BPX + sl * 512:blk * BPX + (sl + 1) * 512]
                        nc.tensor.matmul(dst[:, ch * 1024 + sl * 512:
                                             ch * 1024 + (sl + 1) * 512],
                                         lhsT, rhs, start=start, stop=stop)

            def p1_stats(blk):
                img = blk // 4
                for sl in range(4):
                    nc.vector.bn_stats(
                        out=lnstat[:, img, sl // 2, blk % 4, sl % 2, :],
                        in_=accs[blk][:, sl * 512:(sl + 1) * 512])

            def img_combine_pre(img):
                """DVE part: rhsT[:, img, :] = per-partition (Σmean', ΣE2')."""
                u = rhsT[:, img, 0:1]
                s2 = rhsT[:, img, 1:2]
                # mean'_ch = mean_ch + bv_ch ; E2'_ch = var_ch + mean'^2
                for ch in range(2):
                    nc.vector.bn_aggr(out=mvb[:], in_=lnstat[:, img, ch])
                    mm, vv = mvb[:, 0:1], mvb[:, 1:2]
                    nc.vector.tensor_add(mm, mm, bnpt[:, 4 + ch:5 + ch])
                    nc.vector.tensor_scalar(vv, mm, mm, vv, OP.mult, OP.add)
                    if ch == 0:
                        nc.vector.tensor_copy(u, mm)
                        nc.vector.tensor_copy(s2, vv)
                    else:
                        nc.vector.tensor_add(u, u, mm)
                        nc.vector.tensor_add(s2, s2, vv)

            def img_pcomb(img):
                """PE: reduce rhsT across partitions (broadcast to all)."""
                pc = ps.tile([128, 2048], F32, tag="ps", name=f"pcomb{img}")
                nc.tensor.matmul(pc[:, 0:2], onesM[:], rhsT[:, img, :],
                                 start=True, stop=True)
                nc.vector.tensor_scalar(mE[:, img, :], pc[:, 0:2], 1.0 / C,
                                        None, OP.mult)

            def img_coefs(img):
                m, e2 = mE[:, img, 0:1], mE[:, img, 1:2]
                v = rr[:, img:img + 1]
                nc.vector.tensor_scalar(v, m, m, None, OP.mult)
                nc.vector.tensor_sub(v, e2, v)
                nc.scalar.activation(out=v, in_=v, func=AF.Sqrt, bias=eps128[:])
                nc.vector.reciprocal(out=v, in_=v)              # r = rstd
                # bias per (img, ch) = r * (bv_ch - m)
                for ch in range(2):
                    bb = lbias[:, img, ch:ch + 1]
                    nc.vector.tensor_sub(bb, bnpt[:, 4 + ch:5 + ch], m)
                    nc.vector.tensor_mul(bb, bb, v)

            def p2_block(blk):
                img = blk // 4
                P = ps.tile([128, 2048], F32, tag="ps", name=f"p2_{blk}")
                kc_mms(P, blk, 0, True, False)
                kc_mms(P, blk, 1, False, True)
                ot = outp.tile([128, 2048], F32, tag="ot", name=f"ot{blk}")
                if fast_ln:
                    for ch in range(2):
                        nc.scalar.activation(
                            out=ot[:, ch * 1024:(ch + 1) * 1024],
                            in_=P[:, ch * 1024:(ch + 1) * 1024],
                            func=AF.Prelu, bias=lbias[:, img, ch:ch + 1],
                            scale=rr[:, img:img + 1], alpha=ALPHA)
                else:
                    lo = (blk % 4) * BPX
                    for ch in range(2):
                        seg = ot[:, ch * 1024:(ch + 1) * 1024]
                        nc.scalar.activation(
                            out=seg, in_=P[:, ch * 1024:(ch + 1) * 1024],
                            func=AF.Identity, bias=lbias[:, img, ch:ch + 1],
                            scale=rr[:, img:img + 1])
                        nc.vector.tensor_mul(seg, seg, lngt[:, ch, lo:lo + BPX])
                        nc.vector.tensor_add(seg, seg, lnbt[:, ch, lo:lo + BPX])
                        nc.scalar.activation(out=seg, in_=seg, func=AF.Prelu,
                                             bias=0.0, scale=1.0, alpha=ALPHA)
                q = nc.sync if blk % 2 == 0 else nc.gpsimd
                q.dma_start(out=yout.ap()[:, blk * 2048:(blk + 1) * 2048], in_=ot[:])

            # pass 1 with kc0 prefetch for blocks 0..1 (hides part of the AR)
            for blk in range(2):
                accs[blk] = ps.tile([128, 2048], F32, tag="ps", name=f"p1_{blk}")
                kc_mms(accs[blk], blk, 0, True, False)
            for blk in range(NBLK):
                if blk >= 2:
                    accs[blk] = ps.tile([128, 2048], F32, tag="ps", name=f"p1_{blk}")
                    kc_mms(accs[blk], blk, 0, True, False)
                kc_mms(accs[blk], blk, 1, False, True)
                p1_stats(blk)
                if blk == 3:
                    img_combine_pre(0)
                if blk == 5:
                    img_pcomb(0)
                    img_coefs(0)
            img_combine_pre(1)
            img_pcomb(1)
            img_coefs(1)

            # pass 2: re-run matmuls, fuse LN+leaky straight out of PSUM
            for blk in range(NBLK):
                p2_block(blk)

    nc.compile()
    return nc


def kernel(**inputs):
    global LAST_RESULT
    x = np.ascontiguousarray(np.asarray(inputs["inputs"], dtype=np.float32))
    cbl_w = np.asarray(inputs["cbl_w"], dtype=np.float32)
    bn_gamma = np.asarray(inputs["bn_gamma"], dtype=np.float32)
    bn_beta = np.asarray(inputs["bn_beta"], dtype=np.float32)
    wv = np.asarray(inputs["wv"], dtype=np.float32).reshape(C, C)
    bv = np.asarray(inputs["bv"], dtype=np.float32)
    ln_gamma = np.asarray(inputs["ln_gamma"], dtype=np.float32)
    ln_beta = np.asarray(inputs["ln_beta"], dtype=np.float32)

    fast_ln = bool(np.all(ln_gamma == 1.0) and np.all(ln_beta == 0.0))
    # host-side repack (free for HW time): channel-major, pre-padded input
    xp = np.zeros((NCORES, CIN, BL, HP, WP), np.float32)
    xp[:, :, :, 1:H + 1, 1:W + 1] = (
        x.reshape(NCORES, BL, H, W, CIN).transpose(0, 4, 1, 2, 3))
    xin = np.ascontiguousarray(xp.reshape(NCORES, CIN, BL * HP * WP))
    # conv weights chunk-major: [cin, ch, tap, m]
    cw = np.ascontiguousarray(
        cbl_w.reshape(9, CIN, 2, 128).transpose(1, 2, 0, 3).reshape(CIN, 2304))
    wv_eff = wv + np.eye(C, dtype=np.float32)
    # [i_local, kc, ch, m]
    wvd = np.ascontiguousarray(
        wv_eff.reshape(2, 128, 2, 128).transpose(1, 0, 2, 3).reshape(128, 512))
    colsum = wv_eff.sum(axis=1)          # [256]
    bnp = np.ascontiguousarray(np.stack([
        bn_gamma[0:128], bn_gamma[128:256],
        bn_beta[0:128], bn_beta[128:256],
        bv[0:128], bv[128:256],
        colsum[0:128], colsum[128:256]], axis=1))

    if fast_ln not in _CACHE:
        _CACHE[fast_ln] = _build(fast_ln)
    nc = _CACHE[fast_ln]

    in_maps = []
    for i in range(NCORES):
        m = {"xin": xin[i], "cw": cw, "wvd": wvd, "bnp": bnp}
        if not fast_ln:
            m["lng"] = np.ascontiguousarray(
                ln_gamma.transpose(2, 0, 1).reshape(C, IPIX))
            m["lnb"] = np.ascontiguousarray(
                ln_beta.transpose(2, 0, 1).reshape(C, IPIX))
        in_maps.append(m)

    res = run_bass_kernel_spmd(nc, in_maps, core_ids=list(range(NCORES)))
    LAST_RESULT = res

    out = np.empty((B, H, W, C), np.float32)
    for i in range(NCORES):
        yc = res.results[i]["yout"].reshape(128, 2, 4, 2, 1024)
        # axes: [p, img, blk4, ch, j] -> [img, blk4, j, ch, p]
        img = yc.transpose(1, 2, 4, 3, 0).reshape(BL, H, W, C)
        out[i * BL:(i + 1) * BL] = img
    return out
